# revision 14
# baseline (speedup 1.0000x reference)
"""DeepKMeans (vq_codebook) Trainium2 Bass kernel.

Reference computation (B=16384, IN=1024, D=256, K=1024):
    embeddings  = x @ W_enc + b_enc                         [B, D]
    recon       = embeddings @ W_dec + b_dec                [B, IN]
    dist        = max(|e|^2 + |c|^2 - 2 e.c, 0)             [B, K]
    exps        = exp(-1000 * (dist - min_k dist))          [B, K]
    weighted    = dist * exps / sum_k exps                  [B, K]
    returns (weighted, dist, recon, embeddings, embeddings)

Sharding: data-parallel across 8 NeuronCores, 2048 rows each; weights and
cluster reps replicated. Host pre-transposes each x shard (the PE contracts
over the partition dim) and precomputes -2*C^T and |c|^2.

Matmul precision/speed: the exp(-1000*gap) path amplifies distance error
1000x, so bf16/fp32r are out. Instead every matmul runs as a 3-pass
split-fp16 product (a.b = hi_a.hi_b + lo_a.hi_b + (hi_a*2^-11).(lo_b*2^11),
fp32 PSUM accumulate), which measures at fp32-class accuracy (~7e-7 absmax
on a K=256 dot) at 3 cycles/row vs native fp32's 4. The lo operands are
pre-scaled by 2^11 so they stay in fp16 normal range; the matching hi
operand of the other pass is scaled by 2^-11. x and the weights are split
on the host; E is split on device (3 DVE casts per tile).

Per-core device program (per 512-row block, 4 blocks):
  1.  E_T[d, r] = W_enc^T x^T (W_enc halves stationary), +b_enc on the
      PSUM->SBUF copy (b_enc is per-partition in this layout).
  2.  PE-transpose E_T -> E_nat[r, d]: emb output + |e|^2 via ACT Square
      with fused row-sum accumulation.
  3.  recon = E^T_T W_dec and q = E^T_T (-2 C^T), sharing stationary E
      halves; +b_dec broadcast tile on the PSUM->SBUF copy.
  4.  dist = max((|c|^2 + |e|^2) + q, 0) via scalar_tensor_tensor (matches
      the reference's association) + tensor_scalar_max; row-min reduce.
  5.  exps = ACT Exp(-1000*(dist-min)) with fused row-sum; weighted =
      (dist * (1/sum)) * exps via one scalar_tensor_tensor.
"""

import numpy as np

import concourse.bacc as bacc
import concourse.mybir as mybir
from concourse import masks
from concourse.tile import TileContext
from concourse.bass_utils import run_bass_kernel_spmd

F32 = mybir.dt.float32
F16 = mybir.dt.float16
ALU = mybir.AluOpType
ACTF = mybir.ActivationFunctionType

B, IN, D, K = 16384, 1024, 256, 1024
NCORES = 8
BS = B // NCORES          # rows per core
RB = 512                  # rows per block
NBLK = BS // RB
NDC = D // 128            # embedding-dim chunks (2)
NKC = IN // 128           # input-dim chunks (8)
S = 2.0 ** 11             # lo-operand scale (keeps fp16 normals)

_CACHE = {}


def _build():
    nc = bacc.Bacc("TRN2", target_bir_lowering=False, debug=False)

    x_hi = nc.dram_tensor("x_hi", [IN, BS], F16, kind="ExternalInput")
    x_lo = nc.dram_tensor("x_lo", [IN, BS], F16, kind="ExternalInput")
    we_hi = nc.dram_tensor("we_hi", [IN, D], F16, kind="ExternalInput")
    we_lo = nc.dram_tensor("we_lo", [IN, D], F16, kind="ExternalInput")  # *2^11
    wd_hi = nc.dram_tensor("wd_hi", [D, IN], F16, kind="ExternalInput")
    wd_lo = nc.dram_tensor("wd_lo", [D, IN], F16, kind="ExternalInput")  # *2^11
    ct_hi = nc.dram_tensor("ct_hi", [D, K], F16, kind="ExternalInput")   # -2C^T
    ct_lo = nc.dram_tensor("ct_lo", [D, K], F16, kind="ExternalInput")   # *2^11
    b_enc = nc.dram_tensor("b_enc", [D], F32, kind="ExternalInput")
    b_dec = nc.dram_tensor("b_dec", [IN], F32, kind="ExternalInput")
    csq = nc.dram_tensor("csq", [K], F32, kind="ExternalInput")

    dist_o = nc.dram_tensor("dist_o", [BS, K], F32, kind="ExternalOutput")
    wd_o = nc.dram_tensor("wd_o", [BS, K], F32, kind="ExternalOutput")
    recon_o = nc.dram_tensor("recon_o", [BS, IN], F32, kind="ExternalOutput")
    emb_o = nc.dram_tensor("emb_o", [BS, D], F32, kind="ExternalOutput")

    with TileContext(nc) as tc:
        with (
            tc.tile_pool(name="const", bufs=1) as cpool,
            tc.tile_pool(name="io", bufs=2) as iopool,
            tc.tile_pool(name="work", bufs=3) as wpool,
            tc.tile_pool(name="ps", bufs=8, space="PSUM") as ppool,
        ):
            # ---- replicated constants (load order = consumption order:
            # enc weights first, then RX hi operands, then lo, then misc) --
            weh_sb = cpool.tile([128, NKC, D], F16)
            weh_r = we_hi.rearrange("(kc p) d -> p kc d", p=128)
            nc.sync.dma_start(out=weh_sb[:, 0], in_=weh_r[:, 0])
            nc.sync.dma_start(out=weh_sb[:, 1:], in_=weh_r[:, 1:])
            wel_sb = cpool.tile([128, NKC, D], F16)
            wel_r = we_lo.rearrange("(kc p) d -> p kc d", p=128)
            nc.sync.dma_start(out=wel_sb[:, 0], in_=wel_r[:, 0])
            nc.sync.dma_start(out=wel_sb[:, 1:], in_=wel_r[:, 1:])
            wdh_sb = cpool.tile([128, NDC, IN], F16)
            nc.sync.dma_start(
                out=wdh_sb[:], in_=wd_hi.rearrange("(dc p) n -> p dc n", p=128)
            )
            cth_sb = cpool.tile([128, NDC, K], F16)
            nc.sync.dma_start(
                out=cth_sb[:], in_=ct_hi.rearrange("(dc p) k -> p dc k", p=128)
            )
            wdl_sb = cpool.tile([128, NDC, IN], F16)
            nc.sync.dma_start(
                out=wdl_sb[:], in_=wd_lo.rearrange("(dc p) n -> p dc n", p=128)
            )
            ctl_sb = cpool.tile([128, NDC, K], F16)
            nc.sync.dma_start(
                out=ctl_sb[:], in_=ct_lo.rearrange("(dc p) k -> p dc k", p=128)
            )
            benc_sb = cpool.tile([128, NDC], F32)
            nc.sync.dma_start(
                out=benc_sb[:], in_=b_enc.rearrange("(c p) -> p c", p=128)
            )
            csq_row = cpool.tile([1, K], F32)
            nc.sync.dma_start(out=csq_row[:], in_=csq[None, :])
            csq_bc = cpool.tile([128, K], F32)
            nc.gpsimd.partition_broadcast(csq_bc[:], csq_row[:])
            ident = cpool.tile([128, 128], F32)
            masks.make_identity(nc, ident[:])

            # ---- main loop over 512-row blocks ----------------------------
            for blk in range(NBLK):
                r0 = blk * RB
                xh_sb = iopool.tile([128, NKC, RB], F16, tag="xh")
                xl_sb = iopool.tile([128, NKC, RB], F16, tag="xl")
                xh_r = x_hi.rearrange("(kc p) r -> p kc r", p=128)
                xl_r = x_lo.rearrange("(kc p) r -> p kc r", p=128)
                # kc=0 starter chunks land fast so the first matmul of the
                # block starts early; the rest arrives as one bulk DMA each
                nc.sync.dma_start(out=xh_sb[:, 0], in_=xh_r[:, 0, r0:r0 + RB])
                nc.sync.dma_start(out=xl_sb[:, 0], in_=xl_r[:, 0, r0:r0 + RB])
                nc.sync.dma_start(
                    out=xh_sb[:, 1:], in_=xh_r[:, 1:, r0:r0 + RB]
                )
                nc.sync.dma_start(
                    out=xl_sb[:, 1:], in_=xl_r[:, 1:, r0:r0 + RB]
                )
                # x_hi * 2^-11, for pairing with the 2^11-scaled we_lo
                xhs_sb = iopool.tile([128, NKC, RB], F16, tag="xhs")
                nc.vector.tensor_scalar_mul(
                    out=xhs_sb[:], in0=xh_sb[:], scalar1=1.0 / S
                )

                # encoder: E_T[d, r], 3-pass split accumulation
                et_sb = []
                eh_sb = []
                el_sb = []
                es_sb = []
                for dc in range(NDC):
                    dsl = slice(dc * 128, (dc + 1) * 128)
                    et_ps = ppool.tile([128, RB], F32, tag="bank",
                                       name=f"et_ps{dc}")
                    for kc in range(NKC):
                        nc.tensor.matmul(
                            et_ps[:], lhsT=weh_sb[:, kc, dsl], rhs=xh_sb[:, kc, :],
                            start=(kc == 0), stop=False, skip_group_check=True,
                        )
                        nc.tensor.matmul(
                            et_ps[:], lhsT=weh_sb[:, kc, dsl], rhs=xl_sb[:, kc, :],
                            start=False, stop=False, skip_group_check=True,
                        )
                        nc.tensor.matmul(
                            et_ps[:], lhsT=wel_sb[:, kc, dsl], rhs=xhs_sb[:, kc, :],
                            start=False, stop=(kc == NKC - 1),
                            skip_group_check=True,
                        )
                    e_sb = wpool.tile([128, RB], F32, tag=f"et_sb{dc}",
                                      name=f"et_sb{dc}")
                    nc.vector.tensor_scalar_add(
                        out=e_sb[:], in0=et_ps[:], scalar1=benc_sb[:, dc:dc + 1]
                    )
                    et_sb.append(e_sb)
                    # device-side split of E for the next matmuls
                    ehi = wpool.tile([128, RB], F16, tag=f"ehi{dc}",
                                     name=f"ehi{dc}")
                    nc.scalar.copy(ehi[:], e_sb[:])
                    elo = wpool.tile([128, RB], F16, tag=f"elo{dc}",
                                     name=f"elo{dc}")
                    nc.vector.tensor_sub(elo[:], e_sb[:], ehi[:])
                    ehs = wpool.tile([128, RB], F16, tag=f"ehs{dc}",
                                     name=f"ehs{dc}")
                    nc.scalar.mul(ehs[:], ehi[:], 1.0 / S)
                    eh_sb.append(ehi)
                    el_sb.append(elo)
                    es_sb.append(ehs)

                for rc in range(RB // 128):
                    rows = slice(r0 + rc * 128, r0 + (rc + 1) * 128)
                    rcol = slice(rc * 128, (rc + 1) * 128)

                    # E_nat[r, d] via PE transpose; emb out + |e|^2
                    en_ps = ppool.tile([128, D], F32, tag="bank", name="en_ps")
                    for dc in range(NDC):
                        nc.tensor.transpose(
                            en_ps[:, dc * 128:(dc + 1) * 128],
                            et_sb[dc][:, rcol],
                            ident[:],
                        )
                    e_nat = wpool.tile([128, D], F32, tag="enat")
                    nc.scalar.copy(e_nat[:], en_ps[:])
                    nc.sync.dma_start(out=emb_o[rows, :], in_=e_nat[:])
                    esq = wpool.tile([128, 1], F32, tag="esq")
                    sq_scratch = wpool.tile([128, D], F32, tag="sqs")
                    nc.scalar.activation(
                        sq_scratch[:], en_ps[:], ACTF.Square, accum_out=esq[:]
                    )

                    # decoder + cross matmuls, 3-pass, shared stationaries
                    r_ps = [
                        ppool.tile([128, 512], F32, tag="bank", name=f"rps{n2}")
                        for n2 in range(IN // 512)
                    ]
                    q_ps = [
                        ppool.tile([128, 512], F32, tag="bank", name=f"qps{n2}")
                        for n2 in range(K // 512)
                    ]
                    for dc in range(NDC):
                        first = dc == 0
                        last = dc == NDC - 1
                        for lhsT, wdec, wct, st, sp in (
                            (eh_sb[dc][:, rcol], wdh_sb, cth_sb, first, False),
                            (el_sb[dc][:, rcol], wdh_sb, cth_sb, False, False),
                            (es_sb[dc][:, rcol], wdl_sb, ctl_sb, False, last),
                        ):
                            for n2 in range(IN // 512):
                                nc.tensor.matmul(
                                    r_ps[n2][:], lhsT=lhsT,
                                    rhs=wdec[:, dc, n2 * 512:(n2 + 1) * 512],
                                    start=st, stop=sp, skip_group_check=True,
                                )
                            for n2 in range(K // 512):
                                nc.tensor.matmul(
                                    q_ps[n2][:], lhsT=lhsT,
                                    rhs=wct[:, dc, n2 * 512:(n2 + 1) * 512],
                                    start=st, stop=sp, skip_group_check=True,
                                )

                    # recon epilogue: raw matmul result (b_dec re-applied on
                    # the host iff nonzero); ACT stages PSUM->SBUF
                    for n2 in range(IN // 512):
                        r_sb = wpool.tile([128, 512], F32, tag="rsb")
                        nc.scalar.copy(r_sb[:], r_ps[n2][:])
                        nc.sync.dma_start(
                            out=recon_o[rows, n2 * 512:(n2 + 1) * 512], in_=r_sb[:]
                        )

                    # distances: (csq + esq) + q, then relu, then row-min
                    t_sb = wpool.tile([128, K], F32, tag="tsb")
                    for n2 in range(K // 512):
                        kslc = slice(n2 * 512, (n2 + 1) * 512)
                        nc.vector.scalar_tensor_tensor(
                            out=t_sb[:, kslc], in0=csq_bc[:, kslc], scalar=esq[:],
                            in1=q_ps[n2][:], op0=ALU.add, op1=ALU.add,
                        )
                    dist_sb = wpool.tile([128, K], F32, tag="dist")
                    nc.scalar.activation(dist_sb[:], t_sb[:], ACTF.Relu)
                    minv = wpool.tile([128, 1], F32, tag="minv")
                    # min(relu(t)) == relu(min(t)): reduce pre-relu t in
                    # parallel with the ACT relu, clamp the [128,1] after
                    nc.vector.tensor_reduce(
                        out=minv[:], in_=t_sb[:],
                        axis=mybir.AxisListType.X, op=ALU.min,
                    )
                    nc.vector.tensor_scalar_max(
                        out=minv[:], in0=minv[:], scalar1=0.0
                    )
                    nc.sync.dma_start(out=dist_o[rows, :], in_=dist_sb[:])

                    # softmax-weighted distances
                    gap_sb = wpool.tile([128, K], F32, tag="gap")
                    nc.vector.tensor_scalar_sub(
                        out=gap_sb[:], in0=dist_sb[:], scalar1=minv[:]
                    )
                    exp_sb = wpool.tile([128, K], F32, tag="exp")
                    sume = wpool.tile([128, 1], F32, tag="sume")
                    nc.scalar.activation(
                        exp_sb[:], gap_sb[:], ACTF.Exp,
                        scale=-1000.0, accum_out=sume[:],
                    )
                    recip = wpool.tile([128, 1], F32, tag="recip")
                    nc.vector.reciprocal(recip[:], sume[:])
                    wd_sb = wpool.tile([128, K], F32, tag="wd")
                    nc.vector.scalar_tensor_tensor(
                        out=wd_sb[:], in0=dist_sb[:], scalar=recip[:],
                        in1=exp_sb[:], op0=ALU.mult, op1=ALU.mult,
                    )
                    nc.sync.dma_start(out=wd_o[rows, :], in_=wd_sb[:])

    nc.compile()
    return nc


def get_nc():
    if "nc" not in _CACHE:
        _CACHE["nc"] = _build()
    return _CACHE["nc"]


def _split16(a):
    """fp32 array -> (hi fp16, lo*2^11 fp16)."""
    a = np.asarray(a, dtype=np.float32)
    hi = a.astype(np.float16)
    lo = ((a - hi.astype(np.float32)) * S).astype(np.float16)
    return hi, lo


def make_in_maps(x, W_enc, b_enc, W_dec, b_dec, cluster_reps):
    x = np.asarray(x, dtype=np.float32)
    we_hi, we_lo = _split16(W_enc)
    wd_hi, wd_lo = _split16(W_dec)
    c2t = -2.0 * np.asarray(cluster_reps, dtype=np.float32).T
    ct_hi, ct_lo = _split16(c2t)
    shared = {
        "we_hi": np.ascontiguousarray(we_hi),
        "we_lo": np.ascontiguousarray(we_lo),
        "wd_hi": np.ascontiguousarray(wd_hi),
        "wd_lo": np.ascontiguousarray(wd_lo),
        "ct_hi": np.ascontiguousarray(ct_hi),
        "ct_lo": np.ascontiguousarray(ct_lo),
        "b_enc": np.ascontiguousarray(np.asarray(b_enc, dtype=np.float32)),
        "b_dec": np.ascontiguousarray(np.asarray(b_dec, dtype=np.float32)),
        "csq": np.ascontiguousarray(
            (np.asarray(cluster_reps, dtype=np.float32) ** 2).sum(axis=1)
        ),
    }
    in_maps = []
    for i in range(NCORES):
        xT = np.ascontiguousarray(x[i * BS:(i + 1) * BS].T)
        xh = xT.astype(np.float16)
        xl = (xT - xh.astype(np.float32)).astype(np.float16)  # unscaled lo
        in_maps.append({"x_hi": xh, "x_lo": xl, **shared})
    return in_maps


def run(x, W_enc, b_enc, W_dec, b_dec, cluster_reps, **run_kwargs):
    nc = get_nc()
    in_maps = make_in_maps(x, W_enc, b_enc, W_dec, b_dec, cluster_reps)
    res = run_bass_kernel_spmd(nc, in_maps, list(range(NCORES)), **run_kwargs)
    wd = np.concatenate([r["wd_o"] for r in res.results], axis=0)
    dist = np.concatenate([r["dist_o"] for r in res.results], axis=0)
    recon = np.concatenate([r["recon_o"] for r in res.results], axis=0)
    bd = np.asarray(b_dec, dtype=np.float32)
    if np.any(bd):
        recon = recon + bd[None, :]
    emb = np.concatenate([r["emb_o"] for r in res.results], axis=0)
    return (wd, dist, recon, emb, emb), res


def kernel(x, W_enc, b_enc, W_dec, b_dec, cluster_reps):
    outs, _ = run(x, W_enc, b_enc, W_dec, b_dec, cluster_reps)
    return outs


# revision 15
# speedup vs baseline: 1.0349x; 1.0349x over previous
"""DeepKMeans (vq_codebook) Trainium2 Bass kernel.

Reference computation (B=16384, IN=1024, D=256, K=1024):
    embeddings  = x @ W_enc + b_enc                         [B, D]
    recon       = embeddings @ W_dec + b_dec                [B, IN]
    dist        = max(|e|^2 + |c|^2 - 2 e.c, 0)             [B, K]
    exps        = exp(-1000 * (dist - min_k dist))          [B, K]
    weighted    = dist * exps / sum_k exps                  [B, K]
    returns (weighted, dist, recon, embeddings, embeddings)

Sharding: data-parallel across 8 NeuronCores, 2048 rows each; weights and
cluster reps replicated. Host pre-transposes each x shard (the PE contracts
over the partition dim) and precomputes -2*C^T and |c|^2.

Matmul precision/speed: the exp(-1000*gap) path amplifies distance error
1000x, so bf16/fp32r are out. Instead every matmul runs as a 3-pass
split-fp16 product (a.b = hi_a.hi_b + lo_a.hi_b + (hi_a*2^-11).(lo_b*2^11),
fp32 PSUM accumulate), which measures at fp32-class accuracy (~7e-7 absmax
on a K=256 dot) at 3 cycles/row vs native fp32's 4. The lo operands are
pre-scaled by 2^11 so they stay in fp16 normal range; the matching hi
operand of the other pass is scaled by 2^-11. x and the weights are split
on the host; E is split on device (3 DVE casts per tile).

Per-core device program (per 512-row block, 4 blocks):
  1.  E_T[d, r] = W_enc^T x^T (W_enc halves stationary), +b_enc on the
      PSUM->SBUF copy (b_enc is per-partition in this layout).
  2.  PE-transpose E_T -> E_nat[r, d]: emb output + |e|^2 via ACT Square
      with fused row-sum accumulation.
  3.  recon = E^T_T W_dec and q = E^T_T (-2 C^T), sharing stationary E
      halves; +b_dec broadcast tile on the PSUM->SBUF copy.
  4.  dist = max((|c|^2 + |e|^2) + q, 0) via scalar_tensor_tensor (matches
      the reference's association) + tensor_scalar_max; row-min reduce.
  5.  exps = ACT Exp(-1000*(dist-min)) with fused row-sum; weighted =
      (dist * (1/sum)) * exps via one scalar_tensor_tensor.
"""

import numpy as np

import concourse.bacc as bacc
import concourse.mybir as mybir
from concourse import masks
from concourse.tile import TileContext
from concourse.bass_utils import run_bass_kernel_spmd

F32 = mybir.dt.float32
F16 = mybir.dt.float16
ALU = mybir.AluOpType
ACTF = mybir.ActivationFunctionType

B, IN, D, K = 16384, 1024, 256, 1024
NCORES = 8
BS = B // NCORES          # rows per core
RB = 512                  # rows per block
NBLK = BS // RB
NDC = D // 128            # embedding-dim chunks (2)
NKC = IN // 128           # input-dim chunks (8)
S = 2.0 ** 11             # lo-operand scale (keeps fp16 normals)

_CACHE = {}


def _build():
    nc = bacc.Bacc("TRN2", target_bir_lowering=False, debug=False)

    x_hi = nc.dram_tensor("x_hi", [IN, BS], F16, kind="ExternalInput")
    x_lo = nc.dram_tensor("x_lo", [IN, BS], F16, kind="ExternalInput")
    we_hi = nc.dram_tensor("we_hi", [IN, D], F16, kind="ExternalInput")
    we_lo = nc.dram_tensor("we_lo", [IN, D], F16, kind="ExternalInput")  # *2^11
    wd_hi = nc.dram_tensor("wd_hi", [D, IN], F16, kind="ExternalInput")
    wd_lo = nc.dram_tensor("wd_lo", [D, IN], F16, kind="ExternalInput")  # *2^11
    ct_hi = nc.dram_tensor("ct_hi", [D, K], F16, kind="ExternalInput")   # -2C^T
    ct_lo = nc.dram_tensor("ct_lo", [D, K], F16, kind="ExternalInput")   # *2^11
    b_enc = nc.dram_tensor("b_enc", [D], F32, kind="ExternalInput")
    b_dec = nc.dram_tensor("b_dec", [IN], F32, kind="ExternalInput")
    csq = nc.dram_tensor("csq", [K], F32, kind="ExternalInput")

    dist_o = nc.dram_tensor("dist_o", [BS, K], F32, kind="ExternalOutput")
    wd_o = nc.dram_tensor("wd_o", [BS, K], F32, kind="ExternalOutput")
    recon_o = nc.dram_tensor("recon_o", [BS, IN], F32, kind="ExternalOutput")
    emb_o = nc.dram_tensor("emb_o", [BS, D], F32, kind="ExternalOutput")

    with TileContext(nc) as tc:
        with (
            tc.tile_pool(name="const", bufs=1) as cpool,
            tc.tile_pool(name="io", bufs=2) as iopool,
            tc.tile_pool(name="work", bufs=3) as wpool,
            tc.tile_pool(name="ps", bufs=8, space="PSUM") as ppool,
        ):
            # ---- replicated constants (load order = consumption order:
            # enc weights first, then RX hi operands, then lo, then misc) --
            weh_sb = cpool.tile([128, NKC, D], F16)
            weh_r = we_hi.rearrange("(kc p) d -> p kc d", p=128)
            nc.sync.dma_start(out=weh_sb[:, 0], in_=weh_r[:, 0])
            nc.sync.dma_start(out=weh_sb[:, 1:], in_=weh_r[:, 1:])
            wel_sb = cpool.tile([128, NKC, D], F16)
            wel_r = we_lo.rearrange("(kc p) d -> p kc d", p=128)
            nc.sync.dma_start(out=wel_sb[:, 0], in_=wel_r[:, 0])
            nc.sync.dma_start(out=wel_sb[:, 1:], in_=wel_r[:, 1:])
            xh_r = x_hi.rearrange("(kc p) r -> p kc r", p=128)
            xl_r = x_lo.rearrange("(kc p) r -> p kc r", p=128)

            def load_x(r0):
                xh_sb = iopool.tile([128, NKC, RB], F16, tag="xh", name="xh_sb")
                xl_sb = iopool.tile([128, NKC, RB], F16, tag="xl", name="xl_sb")
                nc.sync.dma_start(out=xh_sb[:, 0], in_=xh_r[:, 0, r0:r0 + RB])
                nc.sync.dma_start(out=xl_sb[:, 0], in_=xl_r[:, 0, r0:r0 + RB])
                nc.sync.dma_start(out=xh_sb[:, 1:], in_=xh_r[:, 1:, r0:r0 + RB])
                nc.sync.dma_start(out=xl_sb[:, 1:], in_=xl_r[:, 1:, r0:r0 + RB])
                return xh_sb, xl_sb

            x0_tiles = load_x(0)

            wdh_sb = cpool.tile([128, NDC, IN], F16)
            nc.sync.dma_start(
                out=wdh_sb[:], in_=wd_hi.rearrange("(dc p) n -> p dc n", p=128)
            )
            cth_sb = cpool.tile([128, NDC, K], F16)
            nc.sync.dma_start(
                out=cth_sb[:], in_=ct_hi.rearrange("(dc p) k -> p dc k", p=128)
            )
            wdl_sb = cpool.tile([128, NDC, IN], F16)
            nc.sync.dma_start(
                out=wdl_sb[:], in_=wd_lo.rearrange("(dc p) n -> p dc n", p=128)
            )
            ctl_sb = cpool.tile([128, NDC, K], F16)
            nc.sync.dma_start(
                out=ctl_sb[:], in_=ct_lo.rearrange("(dc p) k -> p dc k", p=128)
            )
            benc_sb = cpool.tile([128, NDC], F32)
            nc.sync.dma_start(
                out=benc_sb[:], in_=b_enc.rearrange("(c p) -> p c", p=128)
            )
            csq_row = cpool.tile([1, K], F32)
            nc.sync.dma_start(out=csq_row[:], in_=csq[None, :])
            csq_bc = cpool.tile([128, K], F32)
            nc.gpsimd.partition_broadcast(csq_bc[:], csq_row[:])
            ident = cpool.tile([128, 128], F32)
            masks.make_identity(nc, ident[:])

            # ---- main loop over 512-row blocks ----------------------------
            for blk in range(NBLK):
                r0 = blk * RB
                xh_sb, xl_sb = x0_tiles if blk == 0 else load_x(r0)
                # x_hi * 2^-11, for pairing with the 2^11-scaled we_lo
                xhs_sb = iopool.tile([128, NKC, RB], F16, tag="xhs")
                nc.vector.tensor_scalar_mul(
                    out=xhs_sb[:], in0=xh_sb[:], scalar1=1.0 / S
                )

                # encoder: E_T[d, r], 3-pass split accumulation
                et_sb = []
                eh_sb = []
                el_sb = []
                es_sb = []
                for dc in range(NDC):
                    dsl = slice(dc * 128, (dc + 1) * 128)
                    et_ps = ppool.tile([128, RB], F32, tag="bank",
                                       name=f"et_ps{dc}")
                    for kc in range(NKC):
                        nc.tensor.matmul(
                            et_ps[:], lhsT=weh_sb[:, kc, dsl], rhs=xh_sb[:, kc, :],
                            start=(kc == 0), stop=False, skip_group_check=True,
                        )
                        nc.tensor.matmul(
                            et_ps[:], lhsT=weh_sb[:, kc, dsl], rhs=xl_sb[:, kc, :],
                            start=False, stop=False, skip_group_check=True,
                        )
                        nc.tensor.matmul(
                            et_ps[:], lhsT=wel_sb[:, kc, dsl], rhs=xhs_sb[:, kc, :],
                            start=False, stop=(kc == NKC - 1),
                            skip_group_check=True,
                        )
                    e_sb = wpool.tile([128, RB], F32, tag=f"et_sb{dc}",
                                      name=f"et_sb{dc}")
                    nc.vector.tensor_scalar_add(
                        out=e_sb[:], in0=et_ps[:], scalar1=benc_sb[:, dc:dc + 1]
                    )
                    et_sb.append(e_sb)
                    # device-side split of E for the next matmuls
                    ehi = wpool.tile([128, RB], F16, tag=f"ehi{dc}",
                                     name=f"ehi{dc}")
                    nc.scalar.copy(ehi[:], e_sb[:])
                    elo = wpool.tile([128, RB], F16, tag=f"elo{dc}",
                                     name=f"elo{dc}")
                    nc.vector.tensor_sub(elo[:], e_sb[:], ehi[:])
                    ehs = wpool.tile([128, RB], F16, tag=f"ehs{dc}",
                                     name=f"ehs{dc}")
                    nc.scalar.mul(ehs[:], ehi[:], 1.0 / S)
                    eh_sb.append(ehi)
                    el_sb.append(elo)
                    es_sb.append(ehs)

                for rc in range(RB // 128):
                    rows = slice(r0 + rc * 128, r0 + (rc + 1) * 128)
                    rcol = slice(rc * 128, (rc + 1) * 128)

                    # E_nat[r, d] via PE transpose; emb out + |e|^2
                    en_ps = ppool.tile([128, D], F32, tag="bank", name="en_ps")
                    for dc in range(NDC):
                        nc.tensor.transpose(
                            en_ps[:, dc * 128:(dc + 1) * 128],
                            et_sb[dc][:, rcol],
                            ident[:],
                        )
                    e_nat = wpool.tile([128, D], F32, tag="enat")
                    nc.scalar.copy(e_nat[:], en_ps[:])
                    nc.sync.dma_start(out=emb_o[rows, :], in_=e_nat[:])
                    esq = wpool.tile([128, 1], F32, tag="esq")
                    sq_scratch = wpool.tile([128, D], F32, tag="sqs")
                    nc.scalar.activation(
                        sq_scratch[:], en_ps[:], ACTF.Square, accum_out=esq[:]
                    )

                    # decoder + cross matmuls, 3-pass, shared stationaries
                    r_ps = [
                        ppool.tile([128, 512], F32, tag="bank", name=f"rps{n2}")
                        for n2 in range(IN // 512)
                    ]
                    q_ps = [
                        ppool.tile([128, 512], F32, tag="bank", name=f"qps{n2}")
                        for n2 in range(K // 512)
                    ]
                    for dc in range(NDC):
                        first = dc == 0
                        last = dc == NDC - 1
                        for lhsT, wdec, wct, st, sp in (
                            (eh_sb[dc][:, rcol], wdh_sb, cth_sb, first, False),
                            (el_sb[dc][:, rcol], wdh_sb, cth_sb, False, False),
                            (es_sb[dc][:, rcol], wdl_sb, ctl_sb, False, last),
                        ):
                            for n2 in range(IN // 512):
                                nc.tensor.matmul(
                                    r_ps[n2][:], lhsT=lhsT,
                                    rhs=wdec[:, dc, n2 * 512:(n2 + 1) * 512],
                                    start=st, stop=sp, skip_group_check=True,
                                )
                            for n2 in range(K // 512):
                                nc.tensor.matmul(
                                    q_ps[n2][:], lhsT=lhsT,
                                    rhs=wct[:, dc, n2 * 512:(n2 + 1) * 512],
                                    start=st, stop=sp, skip_group_check=True,
                                )

                    # recon epilogue: raw matmul result (b_dec re-applied on
                    # the host iff nonzero); ACT stages PSUM->SBUF
                    for n2 in range(IN // 512):
                        r_sb = wpool.tile([128, 512], F32, tag="rsb")
                        nc.scalar.copy(r_sb[:], r_ps[n2][:])
                        nc.sync.dma_start(
                            out=recon_o[rows, n2 * 512:(n2 + 1) * 512], in_=r_sb[:]
                        )

                    # distances: (csq + esq) + q, then relu, then row-min
                    t_sb = wpool.tile([128, K], F32, tag="tsb")
                    for n2 in range(K // 512):
                        kslc = slice(n2 * 512, (n2 + 1) * 512)
                        nc.vector.scalar_tensor_tensor(
                            out=t_sb[:, kslc], in0=csq_bc[:, kslc], scalar=esq[:],
                            in1=q_ps[n2][:], op0=ALU.add, op1=ALU.add,
                        )
                    dist_sb = wpool.tile([128, K], F32, tag="dist")
                    nc.scalar.activation(dist_sb[:], t_sb[:], ACTF.Relu)
                    minv = wpool.tile([128, 1], F32, tag="minv")
                    # min(relu(t)) == relu(min(t)): reduce pre-relu t in
                    # parallel with the ACT relu, clamp the [128,1] after
                    nc.vector.tensor_reduce(
                        out=minv[:], in_=t_sb[:],
                        axis=mybir.AxisListType.X, op=ALU.min,
                    )
                    nc.vector.tensor_scalar_max(
                        out=minv[:], in0=minv[:], scalar1=0.0
                    )
                    nc.sync.dma_start(out=dist_o[rows, :], in_=dist_sb[:])

                    # softmax-weighted distances
                    gap_sb = wpool.tile([128, K], F32, tag="gap")
                    nc.vector.tensor_scalar_sub(
                        out=gap_sb[:], in0=dist_sb[:], scalar1=minv[:]
                    )
                    exp_sb = wpool.tile([128, K], F32, tag="exp")
                    sume = wpool.tile([128, 1], F32, tag="sume")
                    nc.scalar.activation(
                        exp_sb[:], gap_sb[:], ACTF.Exp,
                        scale=-1000.0, accum_out=sume[:],
                    )
                    recip = wpool.tile([128, 1], F32, tag="recip")
                    nc.vector.reciprocal(recip[:], sume[:])
                    wd_sb = wpool.tile([128, K], F32, tag="wd")
                    nc.vector.scalar_tensor_tensor(
                        out=wd_sb[:], in0=dist_sb[:], scalar=recip[:],
                        in1=exp_sb[:], op0=ALU.mult, op1=ALU.mult,
                    )
                    nc.sync.dma_start(out=wd_o[rows, :], in_=wd_sb[:])

    nc.compile()
    return nc


def get_nc():
    if "nc" not in _CACHE:
        _CACHE["nc"] = _build()
    return _CACHE["nc"]


def _split16(a):
    """fp32 array -> (hi fp16, lo*2^11 fp16)."""
    a = np.asarray(a, dtype=np.float32)
    hi = a.astype(np.float16)
    lo = ((a - hi.astype(np.float32)) * S).astype(np.float16)
    return hi, lo


def make_in_maps(x, W_enc, b_enc, W_dec, b_dec, cluster_reps):
    x = np.asarray(x, dtype=np.float32)
    we_hi, we_lo = _split16(W_enc)
    wd_hi, wd_lo = _split16(W_dec)
    c2t = -2.0 * np.asarray(cluster_reps, dtype=np.float32).T
    ct_hi, ct_lo = _split16(c2t)
    shared = {
        "we_hi": np.ascontiguousarray(we_hi),
        "we_lo": np.ascontiguousarray(we_lo),
        "wd_hi": np.ascontiguousarray(wd_hi),
        "wd_lo": np.ascontiguousarray(wd_lo),
        "ct_hi": np.ascontiguousarray(ct_hi),
        "ct_lo": np.ascontiguousarray(ct_lo),
        "b_enc": np.ascontiguousarray(np.asarray(b_enc, dtype=np.float32)),
        "b_dec": np.ascontiguousarray(np.asarray(b_dec, dtype=np.float32)),
        "csq": np.ascontiguousarray(
            (np.asarray(cluster_reps, dtype=np.float32) ** 2).sum(axis=1)
        ),
    }
    in_maps = []
    for i in range(NCORES):
        xT = np.ascontiguousarray(x[i * BS:(i + 1) * BS].T)
        xh = xT.astype(np.float16)
        xl = (xT - xh.astype(np.float32)).astype(np.float16)  # unscaled lo
        in_maps.append({"x_hi": xh, "x_lo": xl, **shared})
    return in_maps


def run(x, W_enc, b_enc, W_dec, b_dec, cluster_reps, **run_kwargs):
    nc = get_nc()
    in_maps = make_in_maps(x, W_enc, b_enc, W_dec, b_dec, cluster_reps)
    res = run_bass_kernel_spmd(nc, in_maps, list(range(NCORES)), **run_kwargs)
    wd = np.concatenate([r["wd_o"] for r in res.results], axis=0)
    dist = np.concatenate([r["dist_o"] for r in res.results], axis=0)
    recon = np.concatenate([r["recon_o"] for r in res.results], axis=0)
    bd = np.asarray(b_dec, dtype=np.float32)
    if np.any(bd):
        recon = recon + bd[None, :]
    emb = np.concatenate([r["emb_o"] for r in res.results], axis=0)
    return (wd, dist, recon, emb, emb), res


def kernel(x, W_enc, b_enc, W_dec, b_dec, cluster_reps):
    outs, _ = run(x, W_enc, b_enc, W_dec, b_dec, cluster_reps)
    return outs


# revision 16
# speedup vs baseline: 1.0350x; 1.0001x over previous
"""DeepKMeans (vq_codebook) Trainium2 Bass kernel.

Reference computation (B=16384, IN=1024, D=256, K=1024):
    embeddings  = x @ W_enc + b_enc                         [B, D]
    recon       = embeddings @ W_dec + b_dec                [B, IN]
    dist        = max(|e|^2 + |c|^2 - 2 e.c, 0)             [B, K]
    exps        = exp(-1000 * (dist - min_k dist))          [B, K]
    weighted    = dist * exps / sum_k exps                  [B, K]
    returns (weighted, dist, recon, embeddings, embeddings)

Sharding: data-parallel across 8 NeuronCores, 2048 rows each; weights and
cluster reps replicated. Host pre-transposes each x shard (the PE contracts
over the partition dim) and precomputes -2*C^T and |c|^2.

Matmul precision/speed: the exp(-1000*gap) path amplifies distance error
1000x, so bf16/fp32r are out. Instead every matmul runs as a 3-pass
split-fp16 product (a.b = hi_a.hi_b + lo_a.hi_b + (hi_a*2^-11).(lo_b*2^11),
fp32 PSUM accumulate), which measures at fp32-class accuracy (~7e-7 absmax
on a K=256 dot) at 3 cycles/row vs native fp32's 4. The lo operands are
pre-scaled by 2^11 so they stay in fp16 normal range; the matching hi
operand of the other pass is scaled by 2^-11. x and the weights are split
on the host; E is split on device (3 DVE casts per tile).

Per-core device program (per 512-row block, 4 blocks):
  1.  E_T[d, r] = W_enc^T x^T (W_enc halves stationary), +b_enc on the
      PSUM->SBUF copy (b_enc is per-partition in this layout).
  2.  PE-transpose E_T -> E_nat[r, d]: emb output + |e|^2 via ACT Square
      with fused row-sum accumulation.
  3.  recon = E^T_T W_dec and q = E^T_T (-2 C^T), sharing stationary E
      halves; +b_dec broadcast tile on the PSUM->SBUF copy.
  4.  dist = max((|c|^2 + |e|^2) + q, 0) via scalar_tensor_tensor (matches
      the reference's association) + tensor_scalar_max; row-min reduce.
  5.  exps = ACT Exp(-1000*(dist-min)) with fused row-sum; weighted =
      (dist * (1/sum)) * exps via one scalar_tensor_tensor.
"""

import numpy as np

import concourse.bacc as bacc
import concourse.mybir as mybir
from concourse import masks
from concourse.tile import TileContext
from concourse.bass_utils import run_bass_kernel_spmd

F32 = mybir.dt.float32
F16 = mybir.dt.float16
ALU = mybir.AluOpType
ACTF = mybir.ActivationFunctionType

B, IN, D, K = 16384, 1024, 256, 1024
NCORES = 8
BS = B // NCORES          # rows per core
RB = 512                  # rows per block
NBLK = BS // RB
NDC = D // 128            # embedding-dim chunks (2)
NKC = IN // 128           # input-dim chunks (8)
S = 2.0 ** 11             # lo-operand scale (keeps fp16 normals)

_CACHE = {}


def _build():
    nc = bacc.Bacc("TRN2", target_bir_lowering=False, debug=False)

    x_hi = nc.dram_tensor("x_hi", [IN, BS], F16, kind="ExternalInput")
    x_lo = nc.dram_tensor("x_lo", [IN, BS], F16, kind="ExternalInput")
    we_hi = nc.dram_tensor("we_hi", [IN, D], F16, kind="ExternalInput")
    we_lo = nc.dram_tensor("we_lo", [IN, D], F16, kind="ExternalInput")  # *2^11
    wd_hi = nc.dram_tensor("wd_hi", [D, IN], F16, kind="ExternalInput")
    wd_lo = nc.dram_tensor("wd_lo", [D, IN], F16, kind="ExternalInput")  # *2^11
    ct_hi = nc.dram_tensor("ct_hi", [D, K], F16, kind="ExternalInput")   # -2C^T
    ct_lo = nc.dram_tensor("ct_lo", [D, K], F16, kind="ExternalInput")   # *2^11
    b_enc = nc.dram_tensor("b_enc", [D], F32, kind="ExternalInput")
    b_dec = nc.dram_tensor("b_dec", [IN], F32, kind="ExternalInput")
    csq = nc.dram_tensor("csq", [K], F32, kind="ExternalInput")

    dist_o = nc.dram_tensor("dist_o", [BS, K], F32, kind="ExternalOutput")
    wd_o = nc.dram_tensor("wd_o", [BS, K], F32, kind="ExternalOutput")
    recon_o = nc.dram_tensor("recon_o", [BS, IN], F32, kind="ExternalOutput")
    emb_o = nc.dram_tensor("emb_o", [BS, D], F32, kind="ExternalOutput")

    with TileContext(nc) as tc:
        with (
            tc.tile_pool(name="const", bufs=1) as cpool,
            tc.tile_pool(name="io", bufs=2) as iopool,
            tc.tile_pool(name="work", bufs=3) as wpool,
            tc.tile_pool(name="ps", bufs=8, space="PSUM") as ppool,
        ):
            # ---- replicated constants (load order = consumption order:
            # enc weights first, then RX hi operands, then lo, then misc) --
            weh_sb = cpool.tile([128, NKC, D], F16)
            weh_r = we_hi.rearrange("(kc p) d -> p kc d", p=128)
            nc.sync.dma_start(out=weh_sb[:, 0], in_=weh_r[:, 0])
            nc.sync.dma_start(out=weh_sb[:, 1:], in_=weh_r[:, 1:])
            wel_sb = cpool.tile([128, NKC, D], F16)
            wel_r = we_lo.rearrange("(kc p) d -> p kc d", p=128)
            nc.sync.dma_start(out=wel_sb[:, 0], in_=wel_r[:, 0])
            nc.sync.dma_start(out=wel_sb[:, 1:], in_=wel_r[:, 1:])
            xh_r = x_hi.rearrange("(kc p) r -> p kc r", p=128)
            xl_r = x_lo.rearrange("(kc p) r -> p kc r", p=128)

            def load_x(r0):
                xh_sb = iopool.tile([128, NKC, RB], F16, tag="xh", name="xh_sb")
                xl_sb = iopool.tile([128, NKC, RB], F16, tag="xl", name="xl_sb")
                nc.sync.dma_start(out=xh_sb[:, 0], in_=xh_r[:, 0, r0:r0 + RB])
                nc.sync.dma_start(out=xl_sb[:, 0], in_=xl_r[:, 0, r0:r0 + RB])
                nc.sync.dma_start(out=xh_sb[:, 1:], in_=xh_r[:, 1:, r0:r0 + RB])
                nc.sync.dma_start(out=xl_sb[:, 1:], in_=xl_r[:, 1:, r0:r0 + RB])
                return xh_sb, xl_sb

            x0_tiles = load_x(0)

            wdh_sb = cpool.tile([128, NDC, IN], F16)
            nc.sync.dma_start(
                out=wdh_sb[:], in_=wd_hi.rearrange("(dc p) n -> p dc n", p=128)
            )
            cth_sb = cpool.tile([128, NDC, K], F16)
            nc.sync.dma_start(
                out=cth_sb[:], in_=ct_hi.rearrange("(dc p) k -> p dc k", p=128)
            )
            wdl_sb = cpool.tile([128, NDC, IN], F16)
            nc.sync.dma_start(
                out=wdl_sb[:], in_=wd_lo.rearrange("(dc p) n -> p dc n", p=128)
            )
            ctl_sb = cpool.tile([128, NDC, K], F16)
            nc.sync.dma_start(
                out=ctl_sb[:], in_=ct_lo.rearrange("(dc p) k -> p dc k", p=128)
            )
            benc_sb = cpool.tile([128, NDC], F32)
            nc.sync.dma_start(
                out=benc_sb[:], in_=b_enc.rearrange("(c p) -> p c", p=128)
            )
            csq_row = cpool.tile([1, K], F32)
            nc.sync.dma_start(out=csq_row[:], in_=csq[None, :])
            csq_bc = cpool.tile([128, K], F32)
            nc.gpsimd.partition_broadcast(csq_bc[:], csq_row[:])
            ident = cpool.tile([128, 128], F32)
            masks.make_identity(nc, ident[:])

            # ---- main loop over 512-row blocks ----------------------------
            for blk in range(NBLK):
                r0 = blk * RB
                xh_sb, xl_sb = x0_tiles if blk == 0 else load_x(r0)
                # x_hi * 2^-11, for pairing with the 2^11-scaled we_lo
                xhs_sb = iopool.tile([128, NKC, RB], F16, tag="xhs")
                nc.vector.tensor_scalar_mul(
                    out=xhs_sb[:], in0=xh_sb[:], scalar1=1.0 / S
                )

                # encoder: E_T[d, r], 3-pass split accumulation
                et_sb = []
                eh_sb = []
                el_sb = []
                es_sb = []
                for dc in range(NDC):
                    dsl = slice(dc * 128, (dc + 1) * 128)
                    et_ps = ppool.tile([128, RB], F32, tag="bank",
                                       name=f"et_ps{dc}")
                    for kc in range(NKC):
                        nc.tensor.matmul(
                            et_ps[:], lhsT=weh_sb[:, kc, dsl], rhs=xh_sb[:, kc, :],
                            start=(kc == 0), stop=False, skip_group_check=True,
                        )
                        nc.tensor.matmul(
                            et_ps[:], lhsT=weh_sb[:, kc, dsl], rhs=xl_sb[:, kc, :],
                            start=False, stop=False, skip_group_check=True,
                        )
                        nc.tensor.matmul(
                            et_ps[:], lhsT=wel_sb[:, kc, dsl], rhs=xhs_sb[:, kc, :],
                            start=False, stop=(kc == NKC - 1),
                            skip_group_check=True,
                        )
                    e_sb = wpool.tile([128, RB], F32, tag=f"et_sb{dc}",
                                      name=f"et_sb{dc}")
                    nc.vector.tensor_scalar_add(
                        out=e_sb[:], in0=et_ps[:], scalar1=benc_sb[:, dc:dc + 1]
                    )
                    et_sb.append(e_sb)
                    # device-side split of E for the next matmuls
                    ehi = wpool.tile([128, RB], F16, tag=f"ehi{dc}",
                                     name=f"ehi{dc}")
                    nc.scalar.copy(ehi[:], e_sb[:])
                    elo = wpool.tile([128, RB], F16, tag=f"elo{dc}",
                                     name=f"elo{dc}")
                    nc.vector.tensor_sub(elo[:], e_sb[:], ehi[:])
                    ehs = wpool.tile([128, RB], F16, tag=f"ehs{dc}",
                                     name=f"ehs{dc}")
                    nc.scalar.mul(ehs[:], ehi[:], 1.0 / S)
                    eh_sb.append(ehi)
                    el_sb.append(elo)
                    es_sb.append(ehs)

                for rc in range(RB // 128):
                    rows = slice(r0 + rc * 128, r0 + (rc + 1) * 128)
                    rcol = slice(rc * 128, (rc + 1) * 128)

                    # E_nat[r, d] via PE transpose; emb out + |e|^2
                    en_ps = ppool.tile([128, D], F32, tag="bank", name="en_ps")
                    for dc in range(NDC):
                        nc.tensor.transpose(
                            en_ps[:, dc * 128:(dc + 1) * 128],
                            et_sb[dc][:, rcol],
                            ident[:],
                        )
                    e_nat = wpool.tile([128, D], F32, tag="enat")
                    nc.scalar.copy(e_nat[:], en_ps[:])
                    nc.sync.dma_start(out=emb_o[rows, :], in_=e_nat[:])
                    esq = wpool.tile([128, 1], F32, tag="esq")
                    sq_scratch = wpool.tile([128, D], F32, tag="sqs")
                    nc.scalar.activation(
                        sq_scratch[:], en_ps[:], ACTF.Square, accum_out=esq[:]
                    )

                    # decoder + cross matmuls, 3-pass, shared stationaries
                    r_ps = [
                        ppool.tile([128, 512], F32, tag="bank", name=f"rps{n2}")
                        for n2 in range(IN // 512)
                    ]
                    q_ps = [
                        ppool.tile([128, 512], F32, tag="bank", name=f"qps{n2}")
                        for n2 in range(K // 512)
                    ]
                    for dc in range(NDC):
                        first = dc == 0
                        last = dc == NDC - 1
                        for lhsT, wdec, wct, st, sp in (
                            (eh_sb[dc][:, rcol], wdh_sb, cth_sb, first, False),
                            (el_sb[dc][:, rcol], wdh_sb, cth_sb, False, False),
                            (es_sb[dc][:, rcol], wdl_sb, ctl_sb, False, last),
                        ):
                            for n2 in range(IN // 512):
                                nc.tensor.matmul(
                                    r_ps[n2][:], lhsT=lhsT,
                                    rhs=wdec[:, dc, n2 * 512:(n2 + 1) * 512],
                                    start=st, stop=sp, skip_group_check=True,
                                )
                            for n2 in range(K // 512):
                                nc.tensor.matmul(
                                    q_ps[n2][:], lhsT=lhsT,
                                    rhs=wct[:, dc, n2 * 512:(n2 + 1) * 512],
                                    start=st, stop=sp, skip_group_check=True,
                                )

                    # recon epilogue: raw matmul result (b_dec re-applied on
                    # the host iff nonzero); ACT stages PSUM->SBUF
                    for n2 in range(IN // 512):
                        r_sb = wpool.tile([128, 512], F32, tag="rsb")
                        nc.scalar.copy(r_sb[:], r_ps[n2][:])
                        nc.sync.dma_start(
                            out=recon_o[rows, n2 * 512:(n2 + 1) * 512], in_=r_sb[:]
                        )

                    # distances: (csq + esq) + q, then relu, then row-min
                    t_sb = wpool.tile([128, K], F32, tag="tsb")
                    for n2 in range(K // 512):
                        kslc = slice(n2 * 512, (n2 + 1) * 512)
                        nc.vector.scalar_tensor_tensor(
                            out=t_sb[:, kslc], in0=csq_bc[:, kslc], scalar=esq[:],
                            in1=q_ps[n2][:], op0=ALU.add, op1=ALU.add,
                        )
                    dist_sb = wpool.tile([128, K], F32, tag="dist")
                    minv = wpool.tile([128, 2], F32, tag="minv")
                    for n2 in range(K // 512):
                        kslc = slice(n2 * 512, (n2 + 1) * 512)
                        nc.scalar.activation(
                            dist_sb[:, kslc], t_sb[:, kslc], ACTF.Relu
                        )
                        # min(relu(t)) == relu(min(t)): reduce pre-relu t in
                        # parallel with the ACT relu, combine + clamp after
                        nc.vector.tensor_reduce(
                            out=minv[:, n2:n2 + 1], in_=t_sb[:, kslc],
                            axis=mybir.AxisListType.X, op=ALU.min,
                        )
                        nc.sync.dma_start(
                            out=dist_o[rows, kslc], in_=dist_sb[:, kslc]
                        )
                    minc = wpool.tile([128, 1], F32, tag="minc")
                    nc.vector.tensor_reduce(
                        out=minc[:], in_=minv[:],
                        axis=mybir.AxisListType.X, op=ALU.min,
                    )
                    nc.vector.tensor_scalar_max(
                        out=minc[:], in0=minc[:], scalar1=0.0
                    )

                    # softmax-weighted distances, chunked
                    gap_sb = wpool.tile([128, K], F32, tag="gap")
                    exp_sb = wpool.tile([128, K], F32, tag="exp")
                    sume = wpool.tile([128, 2], F32, tag="sume")
                    for n2 in range(K // 512):
                        kslc = slice(n2 * 512, (n2 + 1) * 512)
                        nc.vector.tensor_scalar_sub(
                            out=gap_sb[:, kslc], in0=dist_sb[:, kslc],
                            scalar1=minc[:],
                        )
                        nc.scalar.activation(
                            exp_sb[:, kslc], gap_sb[:, kslc], ACTF.Exp,
                            scale=-1000.0, accum_out=sume[:, n2:n2 + 1],
                        )
                    sumc = wpool.tile([128, 1], F32, tag="sumc")
                    nc.vector.tensor_reduce(
                        out=sumc[:], in_=sume[:],
                        axis=mybir.AxisListType.X, op=ALU.add,
                    )
                    recip = wpool.tile([128, 1], F32, tag="recip")
                    nc.vector.reciprocal(recip[:], sumc[:])
                    wd_sb = wpool.tile([128, K], F32, tag="wd")
                    for n2 in range(K // 512):
                        kslc = slice(n2 * 512, (n2 + 1) * 512)
                        nc.vector.scalar_tensor_tensor(
                            out=wd_sb[:, kslc], in0=dist_sb[:, kslc],
                            scalar=recip[:], in1=exp_sb[:, kslc],
                            op0=ALU.mult, op1=ALU.mult,
                        )
                        nc.sync.dma_start(
                            out=wd_o[rows, kslc], in_=wd_sb[:, kslc]
                        )

    nc.compile()
    return nc


def get_nc():
    if "nc" not in _CACHE:
        _CACHE["nc"] = _build()
    return _CACHE["nc"]


def _split16(a):
    """fp32 array -> (hi fp16, lo*2^11 fp16)."""
    a = np.asarray(a, dtype=np.float32)
    hi = a.astype(np.float16)
    lo = ((a - hi.astype(np.float32)) * S).astype(np.float16)
    return hi, lo


def make_in_maps(x, W_enc, b_enc, W_dec, b_dec, cluster_reps):
    x = np.asarray(x, dtype=np.float32)
    we_hi, we_lo = _split16(W_enc)
    wd_hi, wd_lo = _split16(W_dec)
    c2t = -2.0 * np.asarray(cluster_reps, dtype=np.float32).T
    ct_hi, ct_lo = _split16(c2t)
    shared = {
        "we_hi": np.ascontiguousarray(we_hi),
        "we_lo": np.ascontiguousarray(we_lo),
        "wd_hi": np.ascontiguousarray(wd_hi),
        "wd_lo": np.ascontiguousarray(wd_lo),
        "ct_hi": np.ascontiguousarray(ct_hi),
        "ct_lo": np.ascontiguousarray(ct_lo),
        "b_enc": np.ascontiguousarray(np.asarray(b_enc, dtype=np.float32)),
        "b_dec": np.ascontiguousarray(np.asarray(b_dec, dtype=np.float32)),
        "csq": np.ascontiguousarray(
            (np.asarray(cluster_reps, dtype=np.float32) ** 2).sum(axis=1)
        ),
    }
    in_maps = []
    for i in range(NCORES):
        xT = np.ascontiguousarray(x[i * BS:(i + 1) * BS].T)
        xh = xT.astype(np.float16)
        xl = (xT - xh.astype(np.float32)).astype(np.float16)  # unscaled lo
        in_maps.append({"x_hi": xh, "x_lo": xl, **shared})
    return in_maps


def run(x, W_enc, b_enc, W_dec, b_dec, cluster_reps, **run_kwargs):
    nc = get_nc()
    in_maps = make_in_maps(x, W_enc, b_enc, W_dec, b_dec, cluster_reps)
    res = run_bass_kernel_spmd(nc, in_maps, list(range(NCORES)), **run_kwargs)
    wd = np.concatenate([r["wd_o"] for r in res.results], axis=0)
    dist = np.concatenate([r["dist_o"] for r in res.results], axis=0)
    recon = np.concatenate([r["recon_o"] for r in res.results], axis=0)
    bd = np.asarray(b_dec, dtype=np.float32)
    if np.any(bd):
        recon = recon + bd[None, :]
    emb = np.concatenate([r["emb_o"] for r in res.results], axis=0)
    return (wd, dist, recon, emb, emb), res


def kernel(x, W_enc, b_enc, W_dec, b_dec, cluster_reps):
    outs, _ = run(x, W_enc, b_enc, W_dec, b_dec, cluster_reps)
    return outs
